# revision 35
# baseline (speedup 1.0000x reference)
"""Trainium2 Bass kernel for nn_BlockSparseMLP (MoE gated MLP, E=8, top-2).

Strategy: expert parallelism over 8 NeuronCores. The router matmul
(x @ w_router, 67 MFLOP out of the 206 GFLOP total) plus the top-2
dispatch/gather and the final scatter-add combine run on the host; each
core runs the full gated MLP (gate/up, silu*up, down, weighted by the
routing prob) for the 512 tokens routed to its expert. All matmul
operands are bf16 (cast on host), PSUM accumulation is fp32, so the
weight/activation DMA traffic is half of fp32 (52 MB/core) while the PE
streams at the same 1 column/cycle rate as fp32r (measured 225 ns per
[128x128]x[128x512] matmul). The device program is PE-bound at ~786k
moving columns ~= 350 us; DMA (~190 us) hides entirely under compute.

Per-core layout (capacity B0 = 512 tokens, token-major tiles of 128):
  phase 1 (gate/up): for each of 16 weight chunks (256 cols of I for
    both gate and up), stream w in two [128, 8, 512] half-chunk tiles
    (8 KB/partition contiguous DMA runs); 16-deep PSUM chains with the
    weight [128,128] stationary and xT [128,512] moving; silu(gate)*up
    fused on ACT+DVE into bf16 aT ([I, tokens] layout, SBUF-resident,
    split in two tiles so phase 2's first chain doesn't wait on all of
    phase 1).
  phase 2 (down): w_down streamed as [128, 8, 512] pieces; the first
    output chunk's pieces ride behind the mc=10/12 weight loads, the
    rest stream during phase 2 on the then-idle weight queues. aT
    slices stationary, paired 32-deep chains into [tokens, 512] PSUM
    tiles, scaled by the per-token routing weight, bf16 out.

DMA scheduling: the three host-visible queues (sync/scalar HWDGE +
gpsimd SWDGE) share a ~280 GB/s per-core budget and each queue
completes transfers strictly in issue order, so queue position is used
as the prefetch throttle throughout. The head spreads the first ~6 MB
(mc0 weights fine-grained, x, mc1) across all three queues in
consumption order.
"""

import sys
import functools

sys.path.insert(0, "/opt/trn_rl_repo")

import numpy as np
import ml_dtypes

BF16 = ml_dtypes.bfloat16

T, H, II, E, TOPK = 2048, 2048, 4096, 8, 2
NCORES = 8
B0 = 512         # token capacity per expert (= moving N in phase 1)
NT = B0 // 128   # 4 token tiles
KT = H // 128    # 16 contraction tiles for gate/up
MTI = II // 128  # 32 I tiles
NMC = II // 256  # 16 gate/up weight chunks (256 I-cols each)
KI = II // 128   # 32 contraction tiles for down
NH = H // 512    # 4 output column chunks


@functools.lru_cache(maxsize=2)
def _build(nb1: int = 0):
    """Build the SPMD Bass program (bf16 operands, fp32 PSUM)."""
    assert nb1 == 0
    import concourse.mybir as mybir
    import concourse.tile as tile
    from concourse import bacc

    f32 = mybir.dt.float32
    bf = mybir.dt.bfloat16

    nc = bacc.Bacc(None)
    # DRAM layouts are packed per-partition-contiguous on the host so every
    # DMA moves long (8-16 KB) runs per partition.
    xT = nc.declare_dram_parameter("xT", [128, KT * B0], bf, isOutput=False)
    wgu = nc.declare_dram_parameter("wgu", [NMC * 2, 128, 8 * 512], bf, isOutput=False)
    wd = nc.declare_dram_parameter("wd", [NH * 4, 128, 8 * 512], bf, isOutput=False)
    rw = nc.declare_dram_parameter("rw", [128, NT], f32, isOutput=False)
    dout = nc.declare_dram_parameter("d", [NT, 128, H], bf, isOutput=True)

    SILU = mybir.ActivationFunctionType.Silu

    with tile.TileContext(nc) as tc:
        with (
            tc.tile_pool(name="pers", bufs=1) as pers,
            tc.tile_pool(name="w0pool", bufs=1) as w0pool,
            tc.tile_pool(name="wpool", bufs=6) as wpool,
            tc.tile_pool(name="wdpool", bufs=8) as wdpool,
        ):
            aTt = [pers.tile([128, MTI // 2, B0], bf, name=f"aT{i}") for i in range(2)]
            rwt = pers.tile([128, NT], f32)
            xt = pers.tile([128, KT, B0], bf)

            # Head scheduling: the three DMA queues share a ~280 GB/s core
            # budget and a queue's transfers complete strictly in issue
            # order, so the first ~6 MB (mc0 weights, x, mc1) is spread
            # across all three queues in consumption order. Queue position
            # doubles as the throttle for everything prefetched early.
            w0s = [None] * KT
            for h, eng in ((0, nc.sync), (1, nc.scalar)):
                for k0, nk in ((0, 2), (2, 2), (4, 4)):
                    t = w0pool.tile([128, nk, 512], bf, name=f"w0_{h}_{k0}")
                    eng.dma_start(t[:], wgu[h][:, k0 * 512:(k0 + nk) * 512])
                    for kl in range(nk):
                        w0s[h * 8 + k0 + kl] = t[:, kl, :]
            nc.gpsimd.dma_start(xt[:, :4, :], xT[:, : 4 * B0])
            nc.gpsimd.dma_start(xt[:, 4:8, :], xT[:, 4 * B0: 8 * B0])
            nc.gpsimd.dma_start(xt[:, 8:, :], xT[:, 8 * B0:])

            wb_pre = {}
            for mc, h, eng in ((1, 0, nc.sync), (1, 1, nc.scalar),
                               (2, 0, nc.gpsimd), (2, 1, nc.sync),
                               (3, 0, nc.scalar), (3, 1, nc.gpsimd)):
                wbk = wpool.tile([128, 8, 512], bf, name="wbk", tag="w")
                eng.dma_start(wbk[:], wgu[mc * 2 + h])
                wb_pre[(mc, h)] = wbk
            nc.gpsimd.dma_start(rwt[:], rw[:])

            wdt_all = {}

            def load_wd_piece(nh, h, q, eng):
                wdk = wdpool.tile([128, 8, 512], bf, name="wdk", tag="wd")
                eng.dma_start(wdk[:], wd[nh * 4 + h * 2 + q])
                wdt_all[(nh, h, q)] = wdk

            with (
                tc.tile_pool(name="ps1", bufs=1, space="PSUM") as ps1,
                tc.tile_pool(name="sp", bufs=2) as sp,
            ):
                for mc in range(NMC):
                    if mc == 0:
                        wb = None
                    elif mc < 4:
                        wb = [wb_pre[(mc, 0)], wb_pre[(mc, 1)]]
                    else:
                        wb = []
                        for h in range(2):
                            wbk = wpool.tile([128, 8, 512], bf, name="wbk", tag="w")
                            eng = nc.sync if h == 0 else nc.scalar
                            eng.dma_start(wbk[:], wgu[mc * 2 + h])
                            wb.append(wbk)
                    if mc in (10, 12):
                        # nh=0 down-proj pieces ride behind the mc10/mc12
                        # weight loads; everything later streams in phase 2
                        q = (mc - 10) // 2
                        load_wd_piece(0, 0, q, nc.sync)
                        load_wd_piece(0, 1, q, nc.scalar)
                    pg = [ps1.tile([128, B0], f32, name="pg", tag="pg", bufs=4)
                          for _ in range(2)]
                    pu = [ps1.tile([128, B0], f32, name="pu", tag="pu", bufs=4)
                          for _ in range(2)]

                    for k in range(KT):
                        hh, kl = divmod(k, 8)
                        st = dict(start=(k == 0), stop=(k == KT - 1))
                        if mc == 0:
                            wslice = w0s[k]
                        else:
                            wslice = wb[hh][:, kl, :]
                        # last chunk, last k-step: finish the j=1 chains
                        # first so the phase-boundary silu*up epilogue (the
                        # serial ACT->DVE path phase 2 waits on) starts ~3
                        # matmul slots earlier
                        if mc == NMC - 1 and k == KT - 1:
                            order = ((pg, 1), (pu, 1), (pg, 0), (pu, 0))
                        else:
                            order = ((pg, 0), (pg, 1), (pu, 0), (pu, 1))
                        for arr, j in order:
                            off = 0 if arr is pg else 256
                            nc.tensor.matmul(
                                arr[j][:], wslice[:, off + j * 128:off + (j + 1) * 128],
                                xt[:, k, :], **st
                            )

                    if mc == NMC - 1:
                        # pipeline the final silu*up in half-tiles (j=1
                        # first) to shorten the serial tail phase 2 waits on
                        for j in (1, 0):
                            m = mc * 2 + j
                            sg = sp.tile([128, B0], f32, name="sg", tag="sg")
                            for hf in range(2):
                                cs = slice(hf * 256, (hf + 1) * 256)
                                nc.scalar.activation(sg[:, cs], pg[j][:, cs], SILU)
                                nc.vector.tensor_mul(
                                    aTt[m // 16][:, m % 16, cs], sg[:, cs], pu[j][:, cs])
                    else:
                        for j in range(2):
                            m = mc * 2 + j
                            sg = sp.tile([128, B0], f32, name="sg", tag="sg")
                            nc.scalar.activation(sg[:], pg[j][:], SILU)
                            nc.vector.tensor_mul(aTt[m // 16][:, m % 16, :], sg[:], pu[j][:])

            with (
                tc.tile_pool(name="ps2", bufs=1, space="PSUM") as ps2,
                tc.tile_pool(name="op", bufs=3) as op,
            ):
                for nh in range(NH):
                    # stream the next nh's wd pieces on the now-idle weight
                    # queues; wdpool slot rotation gates the later ones
                    if nh + 1 < NH:
                        for q in range(2):
                            load_wd_piece(nh + 1, 0, q, nc.sync)
                            load_wd_piece(nh + 1, 1, q, nc.scalar)
                    # paired 32-deep chains so each group's outputs drain
                    # while the next group computes; the very last group runs
                    # as single chains with outputs on the idle HWDGE queues,
                    # keeping the post-last-matmul tail to one scale+DMA
                    # without waiting on a gpsimd ring drain
                    last = nh == NH - 1
                    groups = ((0, 1), (2,), (3,)) if last else ((0, 1), (2, 3))
                    for gi, mts in enumerate(groups):
                        pd = [ps2.tile([128, 512], f32, name="pd", tag="pd", bufs=8)
                              for _ in mts]
                        for k in range(KI):
                            h, kl16 = divmod(k, 16)
                            q, kl = divmod(kl16, 8)
                            st = dict(start=(k == 0), stop=(k == KI - 1))
                            for i, mt in enumerate(mts):
                                nc.tensor.matmul(
                                    pd[i][:], aTt[k // 16][:, k % 16, mt * 128:(mt + 1) * 128],
                                    wdt_all[(nh, h, q)][:, kl, :], **st
                                )
                        for i, mt in enumerate(mts):
                            ot = op.tile([128, 512], bf, name="ot", tag="ot")
                            if last and mt == 3:
                                # final tile: scale + DMA in two parallel
                                # halves on independent engines (DVE mul and
                                # ACT Copy-with-scale compute the same thing)
                                # so the post-last-matmul tail is half a
                                # tile's scale plus a 64 KB DMA
                                nc.vector.tensor_scalar_mul(
                                    ot[:, :256], pd[i][:, :256], rwt[:, mt:mt + 1])
                                nc.sync.dma_start(
                                    dout[mt][:, nh * 512:nh * 512 + 256], ot[:, :256])
                                nc.scalar.activation(
                                    ot[:, 256:], pd[i][:, 256:],
                                    mybir.ActivationFunctionType.Copy,
                                    scale=rwt[:, mt:mt + 1])
                                nc.scalar.dma_start(
                                    dout[mt][:, nh * 512 + 256:(nh + 1) * 512], ot[:, 256:])
                            else:
                                nc.vector.tensor_scalar_mul(ot[:], pd[i][:], rwt[:, mt:mt + 1])
                                eng = (nc.sync if mt == 3 else nc.scalar) if last else nc.gpsimd
                                eng.dma_start(dout[mt][:, nh * 512:(nh + 1) * 512], ot[:])

    nc.compile()
    return nc


@functools.lru_cache(maxsize=2)
def _get_exec(nb1: int = 0):
    """Compile the Bass program and return (nc, run_fn) with a cached jit.

    run_fn(in_maps) -> list of per-core {"d": np.ndarray}. Mirrors
    bass2jax.run_bass_via_pjrt's multi-core branch, but keeps the jitted
    function alive across kernel() calls so repeat invocations skip XLA
    and NEFF compilation.
    """
    import jax
    import concourse.mybir as mybir
    from concourse import bass2jax

    nc = _build(nb1)
    bass2jax.install_neuronx_cc_hook()

    partition_name = nc.partition_id_tensor.name if nc.partition_id_tensor else None
    in_names, out_names, out_avals = [], [], []
    zero_out_shapes = []
    for alloc in nc.m.functions[0].allocations:
        if not isinstance(alloc, mybir.MemoryLocationSet):
            continue
        name = alloc.memorylocations[0].name
        if alloc.kind == "ExternalInput":
            if name != partition_name:
                in_names.append(name)
        elif alloc.kind == "ExternalOutput":
            shape = tuple(alloc.tensor_shape)
            dtype = mybir.dt.np(alloc.dtype)
            out_names.append(name)
            out_avals.append(jax.core.ShapedArray(shape, dtype))
            zero_out_shapes.append((shape, dtype))
    n_params = len(in_names)
    n_outs = len(out_names)
    all_names = list(in_names) + list(out_names)
    if partition_name is not None:
        all_names.append(partition_name)
    donate = tuple(range(n_params, n_params + n_outs))

    def _body(*args):
        operands = list(args)
        if partition_name is not None:
            operands.append(bass2jax.partition_id_tensor())
        outs = bass2jax._bass_exec_p.bind(
            *operands,
            out_avals=tuple(out_avals),
            in_names=tuple(all_names),
            out_names=tuple(out_names),
            lowering_input_output_aliases=(),
            sim_require_finite=True,
            sim_require_nnan=True,
            nc=nc,
        )
        return tuple(outs)

    devices = jax.devices()[:NCORES]
    assert len(devices) == NCORES, f"need {NCORES} devices, have {len(jax.devices())}"
    mesh = bass2jax.Mesh(np.asarray(devices), ("core",))
    in_specs = (bass2jax.PartitionSpec("core"),) * (n_params + n_outs)
    out_specs = (bass2jax.PartitionSpec("core"),) * n_outs
    sharded = jax.jit(
        bass2jax.shard_map(
            _body, mesh=mesh, in_specs=in_specs, out_specs=out_specs, check_rep=False
        ),
        donate_argnums=donate,
        keep_unused=True,
    )

    def run_fn(in_maps):
        concat_in = [
            np.concatenate([np.asarray(m[name]) for m in in_maps], axis=0)
            for name in in_names
        ]
        zeros = [
            np.zeros((shape[0] * NCORES,) + shape[1:], dtype)
            for shape, dtype in zero_out_shapes
        ]
        out_arrs = sharded(*concat_in, *zeros)
        results = []
        for c in range(NCORES):
            res = {}
            for i, name in enumerate(out_names):
                arr = np.asarray(out_arrs[i])
                per = arr.shape[0] // NCORES
                res[name] = arr[c * per:(c + 1) * per]
            results.append(res)
        return results

    return nc, run_fn


def _route(x, w_router):
    """Top-2 routing: expert ids + softmax weights, matching jax.lax.top_k
    (descending, ties to the lower index) + jax.nn.softmax."""
    logits = x.astype(np.float64) @ w_router.astype(np.float64)
    top2 = np.argsort(-logits, axis=1, kind="stable")[:, :TOPK]
    vals = np.take_along_axis(logits, top2, 1).astype(np.float32)
    e = np.exp(vals - vals.max(axis=1, keepdims=True))
    w = (e / e.sum(axis=1, keepdims=True)).astype(np.float32)
    return top2, w


def _reference_numpy(x, w_router, w_gate, w_up, w_down):
    """Correct-but-slow dense fallback for shapes the device program doesn't cover."""
    x = x.astype(np.float32)
    logits = x @ w_router.astype(np.float32)
    n_exp = w_gate.shape[0]
    k = min(TOPK, n_exp)
    top = np.argsort(-logits, axis=1, kind="stable")[:, :k]
    vals = np.take_along_axis(logits, top, 1)
    ex = np.exp(vals - vals.max(1, keepdims=True))
    ww = (ex / ex.sum(1, keepdims=True)).astype(np.float32)
    w_dense = np.zeros_like(logits)
    t_ids = np.arange(x.shape[0])[:, None]
    w_dense[t_ids, top] = ww
    out = np.zeros((x.shape[0], w_down.shape[-1]), np.float32)
    for e in range(n_exp):
        g = x @ w_gate[e]
        u = x @ w_up[e]
        a = (g / (1.0 + np.exp(-g))) * u
        out += w_dense[:, e:e + 1] * (a @ w_down[e])
    return out


def _pack_core_inputs(x, wg_e, wu_e, wd_e, toks, ws, nb1=0):
    """Build one core's input map, bf16, packed so each DMA reads long
    contiguous per-partition runs:
      xT  [128, KT*B0]      xT[p, k*B0+t]        = x[tok_t, k*128+p]
      wgu [NMC*2, 128, 8*512]  [mc*2+h, p, kl*512+c] = w{g|u}[(h*8+kl)*128+p,
                               mc*256 + (c%256)]  (c<256 gate, c>=256 up)
      wd  [NH*4, 128, 8*512]   [nh*4+h*2+q, p, kl*512+c] =
                               wd[(h*16+q*8+kl)*128+p, nh*512+c]
    """
    assert nb1 == 0
    n_e = len(toks)
    xg = np.zeros((B0, H), np.float32)
    xg[:n_e] = x[toks]
    # [H, B0] -> (k, p, t) -> [p, k, t]
    xTm = np.ascontiguousarray(
        xg.T.reshape(KT, 128, B0).transpose(1, 0, 2).reshape(128, KT * B0)
    ).astype(BF16)

    wgu = np.empty((NMC, 2, 128, 8, 512), np.float32)
    # wg_e [H, I] -> (h, kl, p, mc, c) -> (mc, h, p, kl, c)
    wgu[..., :256] = wg_e.reshape(2, 8, 128, NMC, 256).transpose(3, 0, 2, 1, 4)
    wgu[..., 256:] = wu_e.reshape(2, 8, 128, NMC, 256).transpose(3, 0, 2, 1, 4)
    wgu = np.ascontiguousarray(wgu.reshape(NMC * 2, 128, 8 * 512)).astype(BF16)

    # wd_e [I, H] -> (h, q, kl, p, nh, c) -> (nh, h, q, p, kl, c)
    wdm = np.ascontiguousarray(
        wd_e.reshape(2, 2, 8, 128, NH, 512).transpose(4, 0, 1, 3, 2, 5)
        .reshape(NH * 4, 128, 8 * 512)
    ).astype(BF16)

    rfull = np.zeros(B0, np.float32)
    rfull[:n_e] = ws
    return {
        "xT": xTm,
        "wgu": wgu,
        "wd": wdm,
        "rw": np.ascontiguousarray(rfull.reshape(NT, 128).T),
    }


def kernel(x, w_router, w_gate, w_up, w_down):
    x = np.ascontiguousarray(np.asarray(x, dtype=np.float32))
    w_router = np.asarray(w_router, dtype=np.float32)
    w_gate = np.ascontiguousarray(np.asarray(w_gate, dtype=np.float32))
    w_up = np.ascontiguousarray(np.asarray(w_up, dtype=np.float32))
    w_down = np.ascontiguousarray(np.asarray(w_down, dtype=np.float32))

    if (x.shape != (T, H) or w_router.shape != (H, E)
            or w_gate.shape != (E, H, II) or w_up.shape != (E, H, II)
            or w_down.shape != (E, II, H)):
        return _reference_numpy(x, w_router, w_gate, w_up, w_down)

    top2, w = _route(x, w_router)
    tok = np.repeat(np.arange(T), TOPK)
    te = top2.ravel()
    tw = w.ravel()
    toks_e, ws_e = [], []
    for e in range(E):
        sel = te == e
        toks_e.append(tok[sel])
        ws_e.append(tw[sel].astype(np.float32))

    # Capacity-factor dispatch: the device program handles up to B0=512
    # tokens per expert (98.5%+ of routed tokens for balanced routing); the
    # rare spill beyond capacity goes through an exact fp32 host path.
    nc, run_fn = _get_exec(0)

    in_maps = [
        _pack_core_inputs(x, w_gate[e], w_up[e], w_down[e],
                          toks_e[e][:B0], ws_e[e][:B0], 0)
        for e in range(E)
    ]

    try:
        results = run_fn(in_maps)
    except Exception:
        import time as _time
        _time.sleep(20)
        results = run_fn(in_maps)

    out = np.zeros((T, H), np.float32)
    for e in range(E):
        n_e = min(len(toks_e[e]), B0)
        d = np.asarray(results[e]["d"]).astype(np.float32).reshape(B0, H)
        out[toks_e[e][:B0]] += d[:n_e]
        spill = toks_e[e][B0:]
        if spill.size:
            xe = x[spill]
            g = xe @ w_gate[e]
            u = xe @ w_up[e]
            a = (g / (1.0 + np.exp(-g))) * u
            out[spill] += (a @ w_down[e]) * ws_e[e][B0:, None]
    return out


# revision 36
# speedup vs baseline: 1.0040x; 1.0040x over previous
"""Trainium2 Bass kernel for nn_BlockSparseMLP (MoE gated MLP, E=8, top-2).

Strategy: expert parallelism over 8 NeuronCores. The router matmul
(x @ w_router, 67 MFLOP out of the 206 GFLOP total) plus the top-2
dispatch/gather and the final scatter-add combine run on the host; each
core runs the full gated MLP (gate/up, silu*up, down, weighted by the
routing prob) for the 512 tokens routed to its expert. All matmul
operands are bf16 (cast on host), PSUM accumulation is fp32, so the
weight/activation DMA traffic is half of fp32 (52 MB/core) while the PE
streams at the same 1 column/cycle rate as fp32r (measured 225 ns per
[128x128]x[128x512] matmul). The device program is PE-bound at ~786k
moving columns ~= 350 us; DMA (~190 us) hides entirely under compute.

Per-core layout (capacity B0 = 512 tokens, token-major tiles of 128):
  phase 1 (gate/up): for each of 16 weight chunks (256 cols of I for
    both gate and up), stream w in two [128, 8, 512] half-chunk tiles
    (8 KB/partition contiguous DMA runs); 16-deep PSUM chains with the
    weight [128,128] stationary and xT [128,512] moving; silu(gate)*up
    fused on ACT+DVE into bf16 aT ([I, tokens] layout, SBUF-resident,
    split in two tiles so phase 2's first chain doesn't wait on all of
    phase 1).
  phase 2 (down): w_down streamed as [128, 8, 512] pieces; the first
    output chunk's pieces ride behind the mc=10/12 weight loads, the
    rest stream during phase 2 on the then-idle weight queues. aT
    slices stationary, paired 32-deep chains into [tokens, 512] PSUM
    tiles, scaled by the per-token routing weight, bf16 out.

DMA scheduling: the three host-visible queues (sync/scalar HWDGE +
gpsimd SWDGE) share a ~280 GB/s per-core budget and each queue
completes transfers strictly in issue order, so queue position is used
as the prefetch throttle throughout. The head spreads the first ~6 MB
(mc0 weights fine-grained, x, mc1) across all three queues in
consumption order.
"""

import sys
import functools

sys.path.insert(0, "/opt/trn_rl_repo")

import numpy as np
import ml_dtypes

BF16 = ml_dtypes.bfloat16

T, H, II, E, TOPK = 2048, 2048, 4096, 8, 2
NCORES = 8
B0 = 512         # token capacity per expert (= moving N in phase 1)
NT = B0 // 128   # 4 token tiles
KT = H // 128    # 16 contraction tiles for gate/up
MTI = II // 128  # 32 I tiles
NMC = II // 256  # 16 gate/up weight chunks (256 I-cols each)
KI = II // 128   # 32 contraction tiles for down
NH = H // 512    # 4 output column chunks


@functools.lru_cache(maxsize=2)
def _build(nb1: int = 0):
    """Build the SPMD Bass program (bf16 operands, fp32 PSUM)."""
    assert nb1 == 0
    import concourse.mybir as mybir
    import concourse.tile as tile
    from concourse import bacc

    f32 = mybir.dt.float32
    bf = mybir.dt.bfloat16

    nc = bacc.Bacc(None)
    # DRAM layouts are packed per-partition-contiguous on the host so every
    # DMA moves long (8-16 KB) runs per partition.
    xT = nc.declare_dram_parameter("xT", [128, KT * B0], bf, isOutput=False)
    wgu = nc.declare_dram_parameter("wgu", [NMC * 2, 128, 8 * 512], bf, isOutput=False)
    wd = nc.declare_dram_parameter("wd", [NH * 4, 128, 8 * 512], bf, isOutput=False)
    rw = nc.declare_dram_parameter("rw", [128, NT], f32, isOutput=False)
    dout = nc.declare_dram_parameter("d", [NT, 128, H], bf, isOutput=True)

    SILU = mybir.ActivationFunctionType.Silu

    with tile.TileContext(nc) as tc:
        with (
            tc.tile_pool(name="pers", bufs=1) as pers,
            tc.tile_pool(name="w0pool", bufs=1) as w0pool,
            tc.tile_pool(name="wpool", bufs=6) as wpool,
            tc.tile_pool(name="wdpool", bufs=8) as wdpool,
        ):
            aTt = [pers.tile([128, MTI // 2, B0], bf, name=f"aT{i}") for i in range(2)]
            rwt = pers.tile([128, NT], f32)
            xt = pers.tile([128, KT, B0], bf)

            # Head scheduling: the three DMA queues share a ~280 GB/s core
            # budget and a queue's transfers complete strictly in issue
            # order, so the first ~6 MB (mc0 weights, x, mc1) is spread
            # across all three queues in consumption order. Queue position
            # doubles as the throttle for everything prefetched early.
            w0s = [None] * KT
            for h, eng in ((0, nc.sync), (1, nc.scalar)):
                for k0, nk in ((0, 2), (2, 2), (4, 4)):
                    t = w0pool.tile([128, nk, 512], bf, name=f"w0_{h}_{k0}")
                    eng.dma_start(t[:], wgu[h][:, k0 * 512:(k0 + nk) * 512])
                    for kl in range(nk):
                        w0s[h * 8 + k0 + kl] = t[:, kl, :]
            nc.gpsimd.dma_start(xt[:, :4, :], xT[:, : 4 * B0])
            nc.gpsimd.dma_start(xt[:, 4:8, :], xT[:, 4 * B0: 8 * B0])
            nc.gpsimd.dma_start(xt[:, 8:, :], xT[:, 8 * B0:])

            wb_pre = {}
            for mc, h, eng in ((1, 0, nc.sync), (1, 1, nc.scalar),
                               (2, 0, nc.gpsimd), (2, 1, nc.sync),
                               (3, 0, nc.scalar), (3, 1, nc.gpsimd)):
                wbk = wpool.tile([128, 8, 512], bf, name="wbk", tag="w")
                eng.dma_start(wbk[:], wgu[mc * 2 + h])
                wb_pre[(mc, h)] = wbk
            nc.gpsimd.dma_start(rwt[:], rw[:])

            wdt_all = {}

            def load_wd_piece(nh, h, q, eng):
                wdk = wdpool.tile([128, 8, 512], bf, name="wdk", tag="wd")
                eng.dma_start(wdk[:], wd[nh * 4 + h * 2 + q])
                wdt_all[(nh, h, q)] = wdk

            with (
                tc.tile_pool(name="ps1", bufs=1, space="PSUM") as ps1,
                tc.tile_pool(name="sp", bufs=2) as sp,
            ):
                for mc in range(NMC):
                    if mc == 0:
                        wb = None
                    elif mc < 4:
                        wb = [wb_pre[(mc, 0)], wb_pre[(mc, 1)]]
                    else:
                        wb = []
                        for h in range(2):
                            wbk = wpool.tile([128, 8, 512], bf, name="wbk", tag="w")
                            eng = nc.sync if h == 0 else nc.scalar
                            eng.dma_start(wbk[:], wgu[mc * 2 + h])
                            wb.append(wbk)
                    if mc in (10, 12):
                        # nh=0 down-proj pieces ride behind the mc10/mc12
                        # weight loads; everything later streams in phase 2
                        q = (mc - 10) // 2
                        load_wd_piece(0, 0, q, nc.sync)
                        load_wd_piece(0, 1, q, nc.scalar)
                    pg = [ps1.tile([128, B0], f32, name="pg", tag="pg", bufs=4)
                          for _ in range(2)]
                    pu = [ps1.tile([128, B0], f32, name="pu", tag="pu", bufs=4)
                          for _ in range(2)]

                    for k in range(KT):
                        hh, kl = divmod(k, 8)
                        st = dict(start=(k == 0), stop=(k == KT - 1))
                        if mc == 0:
                            wslice = w0s[k]
                        else:
                            wslice = wb[hh][:, kl, :]
                        # last chunk, last k-step: finish the j=1 chains
                        # first so the phase-boundary silu*up epilogue (the
                        # serial ACT->DVE path phase 2 waits on) starts ~3
                        # matmul slots earlier
                        if mc == NMC - 1 and k == KT - 1:
                            order = ((pg, 1), (pu, 1), (pg, 0), (pu, 0))
                        else:
                            order = ((pg, 0), (pg, 1), (pu, 0), (pu, 1))
                        for arr, j in order:
                            off = 0 if arr is pg else 256
                            nc.tensor.matmul(
                                arr[j][:], wslice[:, off + j * 128:off + (j + 1) * 128],
                                xt[:, k, :], **st
                            )

                    if mc == NMC - 1:
                        # pipeline the final silu*up in half-tiles (j=1
                        # first) to shorten the serial tail phase 2 waits on
                        for j in (1, 0):
                            m = mc * 2 + j
                            sg = sp.tile([128, B0], f32, name="sg", tag="sg")
                            for hf in range(2):
                                cs = slice(hf * 256, (hf + 1) * 256)
                                nc.scalar.activation(sg[:, cs], pg[j][:, cs], SILU)
                                nc.vector.tensor_mul(
                                    aTt[m // 16][:, m % 16, cs], sg[:, cs], pu[j][:, cs])
                    else:
                        for j in range(2):
                            m = mc * 2 + j
                            sg = sp.tile([128, B0], f32, name="sg", tag="sg")
                            nc.scalar.activation(sg[:], pg[j][:], SILU)
                            nc.vector.tensor_mul(aTt[m // 16][:, m % 16, :], sg[:], pu[j][:])

            with (
                tc.tile_pool(name="ps2", bufs=1, space="PSUM") as ps2,
                tc.tile_pool(name="op", bufs=3) as op,
            ):
                for nh in range(NH):
                    # stream the next nh's wd pieces on the now-idle weight
                    # queues; wdpool slot rotation gates the later ones
                    if nh + 1 < NH:
                        for q in range(2):
                            load_wd_piece(nh + 1, 0, q, nc.sync)
                            load_wd_piece(nh + 1, 1, q, nc.scalar)
                    # paired 32-deep chains so each group's outputs drain
                    # while the next group computes; the very last group runs
                    # as single chains with outputs on the idle HWDGE queues,
                    # keeping the post-last-matmul tail to one scale+DMA
                    # without waiting on a gpsimd ring drain
                    last = nh == NH - 1
                    groups = ((0, 1), (2,), (3,)) if last else ((0, 1), (2, 3))
                    for gi, mts in enumerate(groups):
                        pd = [ps2.tile([128, 512], f32, name="pd", tag="pd", bufs=8)
                              for _ in mts]
                        for k in range(KI):
                            h, kl16 = divmod(k, 16)
                            q, kl = divmod(kl16, 8)
                            st = dict(start=(k == 0), stop=(k == KI - 1))
                            for i, mt in enumerate(mts):
                                nc.tensor.matmul(
                                    pd[i][:], aTt[k // 16][:, k % 16, mt * 128:(mt + 1) * 128],
                                    wdt_all[(nh, h, q)][:, kl, :], **st
                                )
                        for i, mt in enumerate(mts):
                            ot = op.tile([128, 512], bf, name="ot", tag="ot")
                            if last and mt == 3:
                                # final tile: scale in two halves on the
                                # vector engine (the scalar engine sits in a
                                # semaphore wait ~0.5 us past the last matmul,
                                # so DVE-serial beats a DVE/ACT split) with
                                # each half's 64 KB DMA on its own idle queue
                                nc.vector.tensor_scalar_mul(
                                    ot[:, :256], pd[i][:, :256], rwt[:, mt:mt + 1])
                                nc.sync.dma_start(
                                    dout[mt][:, nh * 512:nh * 512 + 256], ot[:, :256])
                                nc.vector.tensor_scalar_mul(
                                    ot[:, 256:], pd[i][:, 256:], rwt[:, mt:mt + 1])
                                nc.scalar.dma_start(
                                    dout[mt][:, nh * 512 + 256:(nh + 1) * 512], ot[:, 256:])
                            else:
                                nc.vector.tensor_scalar_mul(ot[:], pd[i][:], rwt[:, mt:mt + 1])
                                eng = (nc.sync if mt == 3 else nc.scalar) if last else nc.gpsimd
                                eng.dma_start(dout[mt][:, nh * 512:(nh + 1) * 512], ot[:])

    nc.compile()
    return nc


@functools.lru_cache(maxsize=2)
def _get_exec(nb1: int = 0):
    """Compile the Bass program and return (nc, run_fn) with a cached jit.

    run_fn(in_maps) -> list of per-core {"d": np.ndarray}. Mirrors
    bass2jax.run_bass_via_pjrt's multi-core branch, but keeps the jitted
    function alive across kernel() calls so repeat invocations skip XLA
    and NEFF compilation.
    """
    import jax
    import concourse.mybir as mybir
    from concourse import bass2jax

    nc = _build(nb1)
    bass2jax.install_neuronx_cc_hook()

    partition_name = nc.partition_id_tensor.name if nc.partition_id_tensor else None
    in_names, out_names, out_avals = [], [], []
    zero_out_shapes = []
    for alloc in nc.m.functions[0].allocations:
        if not isinstance(alloc, mybir.MemoryLocationSet):
            continue
        name = alloc.memorylocations[0].name
        if alloc.kind == "ExternalInput":
            if name != partition_name:
                in_names.append(name)
        elif alloc.kind == "ExternalOutput":
            shape = tuple(alloc.tensor_shape)
            dtype = mybir.dt.np(alloc.dtype)
            out_names.append(name)
            out_avals.append(jax.core.ShapedArray(shape, dtype))
            zero_out_shapes.append((shape, dtype))
    n_params = len(in_names)
    n_outs = len(out_names)
    all_names = list(in_names) + list(out_names)
    if partition_name is not None:
        all_names.append(partition_name)
    donate = tuple(range(n_params, n_params + n_outs))

    def _body(*args):
        operands = list(args)
        if partition_name is not None:
            operands.append(bass2jax.partition_id_tensor())
        outs = bass2jax._bass_exec_p.bind(
            *operands,
            out_avals=tuple(out_avals),
            in_names=tuple(all_names),
            out_names=tuple(out_names),
            lowering_input_output_aliases=(),
            sim_require_finite=True,
            sim_require_nnan=True,
            nc=nc,
        )
        return tuple(outs)

    devices = jax.devices()[:NCORES]
    assert len(devices) == NCORES, f"need {NCORES} devices, have {len(jax.devices())}"
    mesh = bass2jax.Mesh(np.asarray(devices), ("core",))
    in_specs = (bass2jax.PartitionSpec("core"),) * (n_params + n_outs)
    out_specs = (bass2jax.PartitionSpec("core"),) * n_outs
    sharded = jax.jit(
        bass2jax.shard_map(
            _body, mesh=mesh, in_specs=in_specs, out_specs=out_specs, check_rep=False
        ),
        donate_argnums=donate,
        keep_unused=True,
    )

    def run_fn(in_maps):
        concat_in = [
            np.concatenate([np.asarray(m[name]) for m in in_maps], axis=0)
            for name in in_names
        ]
        zeros = [
            np.zeros((shape[0] * NCORES,) + shape[1:], dtype)
            for shape, dtype in zero_out_shapes
        ]
        out_arrs = sharded(*concat_in, *zeros)
        results = []
        for c in range(NCORES):
            res = {}
            for i, name in enumerate(out_names):
                arr = np.asarray(out_arrs[i])
                per = arr.shape[0] // NCORES
                res[name] = arr[c * per:(c + 1) * per]
            results.append(res)
        return results

    return nc, run_fn


def _route(x, w_router):
    """Top-2 routing: expert ids + softmax weights, matching jax.lax.top_k
    (descending, ties to the lower index) + jax.nn.softmax."""
    logits = x.astype(np.float64) @ w_router.astype(np.float64)
    top2 = np.argsort(-logits, axis=1, kind="stable")[:, :TOPK]
    vals = np.take_along_axis(logits, top2, 1).astype(np.float32)
    e = np.exp(vals - vals.max(axis=1, keepdims=True))
    w = (e / e.sum(axis=1, keepdims=True)).astype(np.float32)
    return top2, w


def _reference_numpy(x, w_router, w_gate, w_up, w_down):
    """Correct-but-slow dense fallback for shapes the device program doesn't cover."""
    x = x.astype(np.float32)
    logits = x @ w_router.astype(np.float32)
    n_exp = w_gate.shape[0]
    k = min(TOPK, n_exp)
    top = np.argsort(-logits, axis=1, kind="stable")[:, :k]
    vals = np.take_along_axis(logits, top, 1)
    ex = np.exp(vals - vals.max(1, keepdims=True))
    ww = (ex / ex.sum(1, keepdims=True)).astype(np.float32)
    w_dense = np.zeros_like(logits)
    t_ids = np.arange(x.shape[0])[:, None]
    w_dense[t_ids, top] = ww
    out = np.zeros((x.shape[0], w_down.shape[-1]), np.float32)
    for e in range(n_exp):
        g = x @ w_gate[e]
        u = x @ w_up[e]
        a = (g / (1.0 + np.exp(-g))) * u
        out += w_dense[:, e:e + 1] * (a @ w_down[e])
    return out


def _pack_core_inputs(x, wg_e, wu_e, wd_e, toks, ws, nb1=0):
    """Build one core's input map, bf16, packed so each DMA reads long
    contiguous per-partition runs:
      xT  [128, KT*B0]      xT[p, k*B0+t]        = x[tok_t, k*128+p]
      wgu [NMC*2, 128, 8*512]  [mc*2+h, p, kl*512+c] = w{g|u}[(h*8+kl)*128+p,
                               mc*256 + (c%256)]  (c<256 gate, c>=256 up)
      wd  [NH*4, 128, 8*512]   [nh*4+h*2+q, p, kl*512+c] =
                               wd[(h*16+q*8+kl)*128+p, nh*512+c]
    """
    assert nb1 == 0
    n_e = len(toks)
    xg = np.zeros((B0, H), np.float32)
    xg[:n_e] = x[toks]
    # [H, B0] -> (k, p, t) -> [p, k, t]
    xTm = np.ascontiguousarray(
        xg.T.reshape(KT, 128, B0).transpose(1, 0, 2).reshape(128, KT * B0)
    ).astype(BF16)

    wgu = np.empty((NMC, 2, 128, 8, 512), np.float32)
    # wg_e [H, I] -> (h, kl, p, mc, c) -> (mc, h, p, kl, c)
    wgu[..., :256] = wg_e.reshape(2, 8, 128, NMC, 256).transpose(3, 0, 2, 1, 4)
    wgu[..., 256:] = wu_e.reshape(2, 8, 128, NMC, 256).transpose(3, 0, 2, 1, 4)
    wgu = np.ascontiguousarray(wgu.reshape(NMC * 2, 128, 8 * 512)).astype(BF16)

    # wd_e [I, H] -> (h, q, kl, p, nh, c) -> (nh, h, q, p, kl, c)
    wdm = np.ascontiguousarray(
        wd_e.reshape(2, 2, 8, 128, NH, 512).transpose(4, 0, 1, 3, 2, 5)
        .reshape(NH * 4, 128, 8 * 512)
    ).astype(BF16)

    rfull = np.zeros(B0, np.float32)
    rfull[:n_e] = ws
    return {
        "xT": xTm,
        "wgu": wgu,
        "wd": wdm,
        "rw": np.ascontiguousarray(rfull.reshape(NT, 128).T),
    }


def kernel(x, w_router, w_gate, w_up, w_down):
    x = np.ascontiguousarray(np.asarray(x, dtype=np.float32))
    w_router = np.asarray(w_router, dtype=np.float32)
    w_gate = np.ascontiguousarray(np.asarray(w_gate, dtype=np.float32))
    w_up = np.ascontiguousarray(np.asarray(w_up, dtype=np.float32))
    w_down = np.ascontiguousarray(np.asarray(w_down, dtype=np.float32))

    if (x.shape != (T, H) or w_router.shape != (H, E)
            or w_gate.shape != (E, H, II) or w_up.shape != (E, H, II)
            or w_down.shape != (E, II, H)):
        return _reference_numpy(x, w_router, w_gate, w_up, w_down)

    top2, w = _route(x, w_router)
    tok = np.repeat(np.arange(T), TOPK)
    te = top2.ravel()
    tw = w.ravel()
    toks_e, ws_e = [], []
    for e in range(E):
        sel = te == e
        toks_e.append(tok[sel])
        ws_e.append(tw[sel].astype(np.float32))

    # Capacity-factor dispatch: the device program handles up to B0=512
    # tokens per expert (98.5%+ of routed tokens for balanced routing); the
    # rare spill beyond capacity goes through an exact fp32 host path.
    nc, run_fn = _get_exec(0)

    in_maps = [
        _pack_core_inputs(x, w_gate[e], w_up[e], w_down[e],
                          toks_e[e][:B0], ws_e[e][:B0], 0)
        for e in range(E)
    ]

    try:
        results = run_fn(in_maps)
    except Exception:
        import time as _time
        _time.sleep(20)
        results = run_fn(in_maps)

    out = np.zeros((T, H), np.float32)
    for e in range(E):
        n_e = min(len(toks_e[e]), B0)
        d = np.asarray(results[e]["d"]).astype(np.float32).reshape(B0, H)
        out[toks_e[e][:B0]] += d[:n_e]
        spill = toks_e[e][B0:]
        if spill.size:
            xe = x[spill]
            g = xe @ w_gate[e]
            u = xe @ w_up[e]
            a = (g / (1.0 + np.exp(-g))) * u
            out[spill] += (a @ w_down[e]) * ws_e[e][B0:, None]
    return out
